# revision 1
# baseline (speedup 1.0000x reference)
"""ColBERT MaxSim kernel for Trainium2 (8 NeuronCores, Bass/Tile).

Computes, for full inputs:
    q  = l2norm(Q_hid @ W.T)                       (B, L_q, F)
    d  = l2norm(D_hid @ W.T) * d_mask              (B*N, L_d, F)
    sim[b,n,q,t] = <q[b,q], d[b*N+n,t]>, masked -> -inf
    out[b,n] = sum_q max_t sim                     (B, N)

Strategy: data-parallel over the batch. Core c owns batches [4c, 4c+4)
(query side) and doc rows [16c, 16c+16) of D_hid/d_mask; W replicated.
Host-side marshalling transposes Q/D/W so the hidden dim (768) lands on
SBUF partitions (the TensorE contraction axis); all math runs on-device.
"""

import os
import sys

for _p in ("/opt/trn_rl_repo", "/root/.axon_site/_ro/trn_rl_repo"):
    if os.path.isdir(_p) and _p not in sys.path:
        sys.path.insert(0, _p)
        break

import numpy as np

B, N_P, L_Q, L_D, HID, DIM = 32, 4, 64, 1024, 768, 128
N_CORES = 8
B_PER = B // N_CORES                # 4 batches per core
DOCS_PER = (B * N_P) // N_CORES     # 16 docs per core
HC = HID // 128                     # 6 hidden chunks of 128
THALF = 512                         # tokens per PSUM tile (fp32 moving-op limit)
NEG_BIG = -1.0e30

_CACHE = {}


def _build_bass():
    import concourse.bacc as bacc
    import concourse.tile as tile
    from concourse import mybir

    f32 = mybir.dt.float32
    u8 = mybir.dt.uint8
    X = mybir.AxisListType.X
    MAX = mybir.AluOpType.max

    f32r = mybir.dt.float32r
    nc = bacc.Bacc(None, target_bir_lowering=False, debug=False)

    QT = nc.dram_tensor("QT", [128, B_PER, HC, L_Q], f32, kind="ExternalInput")
    DT = nc.dram_tensor(
        "DT", [DOCS_PER // 2, 128, 2, HC, L_D], f32r, kind="ExternalInput"
    )
    WT = nc.dram_tensor("WT", [128, HC, DIM], f32, kind="ExternalInput")
    MK = nc.dram_tensor("MK", [DOCS_PER, L_D], u8, kind="ExternalInput")
    OUT = nc.dram_tensor("out", [1, DOCS_PER], f32, kind="ExternalOutput")

    with tile.TileContext(nc) as tc:
        with (
            tc.tile_pool(name="const", bufs=1) as constp,
            tc.tile_pool(name="dtp", bufs=5) as dtp,
            tc.tile_pool(name="work", bufs=3) as work,
            tc.tile_pool(name="small", bufs=2) as small,
            tc.tile_pool(name="once", bufs=1) as once,
            tc.tile_pool(name="psum", bufs=1, space="PSUM") as pp,
            tc.tile_pool(name="psum_pd", bufs=4, space="PSUM") as ppd,
            tc.tile_pool(name="psum_sim", bufs=3, space="PSUM") as pps,
        ):
            # --- constants ---------------------------------------------------
            wt_sb = constp.tile([128, HC, DIM], f32)
            nc.scalar.dma_start(out=wt_sb[:], in_=WT[:])
            # f32r copy of W for the doc-side matmuls (DVE rounds on write)
            wt_r = constp.tile([128, HC, DIM], f32r)
            nc.vector.tensor_copy(wt_r[:], wt_sb[:])
            ones_f32 = constp.tile([128, 128], f32)
            nc.vector.memset(ones_f32[:], 1.0)
            ones_128 = constp.tile([128, 128], f32r)
            nc.vector.tensor_copy(ones_128[:], ones_f32[:])
            ones_qx1 = constp.tile([L_Q, 1], f32)
            nc.vector.memset(ones_qx1[:], 1.0)

            # --- mask -> additive bias rows (16, 1024) + one-hot selector ----
            # bias16[d, t] = 0 where unmasked, -1e30 where masked. Added into
            # the sim PSUM via a k=16 matmul with a one-hot column selector.
            m8 = once.tile([DOCS_PER, L_D], u8, tag="m8")
            nc.scalar.dma_start(out=m8[:], in_=MK[:])
            biasA = once.tile([DOCS_PER, L_D], f32, tag="biasA")
            nc.vector.tensor_scalar_mul(biasA[:], m8[:], -NEG_BIG)
            bias16 = constp.tile([DOCS_PER, L_D], f32r)
            nc.vector.tensor_scalar_add(bias16[:], biasA[:], NEG_BIG)
            eye_np = np.kron(np.eye(DOCS_PER, dtype=np.float32), np.ones((1, L_Q), np.float32))
            eye_dram = nc.inline_tensor(eye_np, name="eye_rep_const")
            eye8 = once.tile([DOCS_PER, DOCS_PER * L_Q], f32, tag="eye8")
            nc.scalar.dma_start(out=eye8[:], in_=eye_dram[:])
            eye_rep = constp.tile([DOCS_PER, DOCS_PER * L_Q], f32r)
            nc.vector.tensor_copy(eye_rep[:], eye8[:])

            # --- queries: project + l2-normalize -> qnT_all[f, b*64+q] -------
            qnT_all = constp.tile([128, B_PER * L_Q], f32r)
            qt_in = once.tile([128, B_PER, HC, L_Q], f32, tag="qt_in")
            nc.scalar.dma_start(out=qt_in[:], in_=QT[:])
            for b in range(B_PER):
                psq = ppd.tile([128, L_Q], f32, tag="pd")
                for c in range(HC):
                    nc.tensor.matmul(
                        psq[:], wt_sb[:, c, :], qt_in[:, b, c, :],
                        start=(c == 0), stop=(c == HC - 1),
                    )
                sqq = small.tile([128, L_Q], f32r, tag="sqq")
                nc.scalar.square(sqq[:], psq[:])
                pssq = pp.tile([128, L_Q], f32, tag="ps2")
                nc.tensor.matmul(pssq[:], ones_128[:], sqq[:], start=True, stop=True)
                rsqq = small.tile([128, L_Q], f32, tag="rsqq")
                nc.scalar.activation(
                    rsqq[:], pssq[:],
                    mybir.ActivationFunctionType.Abs_reciprocal_sqrt,
                )
                nc.vector.tensor_mul(
                    qnT_all[:, b * L_Q : (b + 1) * L_Q], psq[:], rsqq[:]
                )

            # --- docs: stream, project, normalize, maxsim --------------------
            results = constp.tile([L_Q, DOCS_PER], f32)
            for d in range(DOCS_PER):
                j, i = divmod(d, 2)
                dma_eng = nc.sync if d % 2 == 0 else nc.gpsimd
                dt_in = dtp.tile([128, HC, L_D], f32r)
                if d >= DOCS_PER - 2:
                    # stream each queue's final doc in halves so the last
                    # tiles' compute overlaps the trailing transfer
                    dma_eng.dma_start(
                        out=dt_in[:, :, :THALF], in_=DT[j][:, i, :, :THALF]
                    )
                    dma_eng.dma_start(
                        out=dt_in[:, :, THALF:], in_=DT[j][:, i, :, THALF:]
                    )
                else:
                    dma_eng.dma_start(out=dt_in[:], in_=DT[j][:, i])
                if True:
                    b = d // N_P
                    # smaller tiles on the final docs: shorter dependency
                    # chains, so the post-stream drain is shorter
                    nt = 4 if d >= DOCS_PER - 2 else 2
                    tsz = L_D // nt
                    mxp = small.tile([L_Q, 4], f32, tag="mxp")
                    for ti in range(nt):
                        t0 = ti * tsz
                        pd = ppd.tile([128, tsz], f32, tag="pd")
                        for c in range(HC):
                            nc.tensor.matmul(
                                pd[:], wt_r[:, c, :],
                                dt_in[:, c, t0 : t0 + tsz],
                                start=(c == 0), stop=(c == HC - 1),
                            )
                        sq = work.tile([128, tsz], f32r, tag="sq")
                        nc.scalar.square(sq[:], pd[:])
                        pssq = pp.tile([128, tsz], f32, tag="ps2")
                        nc.tensor.matmul(
                            pssq[:], ones_128[:], sq[:], start=True, stop=True
                        )
                        rsq = work.tile([128, tsz], f32, tag="rsq")
                        nc.scalar.activation(
                            rsq[:], pssq[:],
                            mybir.ActivationFunctionType.Abs_reciprocal_sqrt,
                        )
                        dn = work.tile([128, tsz], f32r, tag="dn")
                        nc.vector.tensor_mul(dn[:], pd[:], rsq[:])

                        ps = pps.tile([L_Q, tsz], f32, tag="psim")
                        nc.tensor.matmul(
                            ps[:], qnT_all[:, b * L_Q : (b + 1) * L_Q], dn[:],
                            start=True, stop=False,
                        )
                        nc.tensor.matmul(
                            ps[:], eye_rep[:, d * L_Q : (d + 1) * L_Q],
                            bias16[:, t0 : t0 + tsz],
                            start=False, stop=True,
                        )
                        nc.vector.tensor_reduce(mxp[:, ti : ti + 1], ps[:], X, MAX)
                    nc.vector.tensor_reduce(
                        results[:, d : d + 1], mxp[:, :nt], X, MAX
                    )

            # --- sum over queries -> (1, 16) ---------------------------------
            pout = pps.tile([1, DOCS_PER], f32, tag="psim")
            nc.tensor.matmul(pout[:], ones_qx1[:], results[:], start=True, stop=True)
            out_sb = constp.tile([1, DOCS_PER], f32)
            nc.vector.tensor_copy(out_sb[:], pout[:])
            nc.sync.dma_start(out=OUT[:], in_=out_sb[:])

    nc.compile()
    return nc


def _get_nc():
    if "nc" not in _CACHE:
        _CACHE["nc"] = _build_bass()
    return _CACHE["nc"]


def _round_f32r(a):
    """Round fp32 array to the PE's FP32R format (RNE to 12 mantissa bits).
    Bit-exact vs libwalrus fp32_to_fp32r. Required: FP32R matmul operands
    must be pre-rounded; the D stream is rounded host-side since an on-device
    rounding pass over 48 MB/core would burn the DVE budget."""
    bits = np.ascontiguousarray(a, dtype=np.float32).view(np.uint32)
    lsb = (bits >> np.uint32(12)) & np.uint32(1)
    r = bits + np.uint32(0x7FF) + lsb
    return (r & np.uint32(0xFFFFF000)).view(np.float32)


def _make_in_maps(Q_hid, D_hid, W, d_mask):
    # Pack into the exact partition-major SBUF layouts so every device DMA is
    # a fully contiguous per-partition descriptor (h = c*128 + p throughout).
    Wp = np.asarray(W, dtype=np.float32)               # (DIM=128 f, HID=768 h)
    WT = np.ascontiguousarray(
        Wp.reshape(DIM, HC, 128).transpose(2, 1, 0)    # (p, c, f)
    )
    in_maps = []
    for c in range(N_CORES):
        qs = np.asarray(Q_hid[B_PER * c : B_PER * (c + 1)], dtype=np.float32)
        ds = np.asarray(D_hid[DOCS_PER * c : DOCS_PER * (c + 1)], dtype=np.float32)
        # qs: (b, q, h) -> (p, b, c, q)
        QTp = np.ascontiguousarray(
            qs.reshape(B_PER, L_Q, HC, 128).transpose(3, 0, 2, 1)
        )
        # ds: (d=2j+i, t, h) -> (j, p, i, c, t): doc pairs, partition-major
        DTp = _round_f32r(np.ascontiguousarray(
            ds.reshape(DOCS_PER // 2, 2, L_D, HC, 128).transpose(0, 4, 1, 3, 2)
        ))
        in_maps.append(
            {
                "QT": QTp,
                "DT": DTp,
                "WT": WT,
                "MK": np.asarray(
                    d_mask[DOCS_PER * c : DOCS_PER * (c + 1)], dtype=np.uint8
                ),
            }
        )
    return in_maps


def run_spmd(Q_hid, D_hid, W, d_mask, trace=False, tmpdir=None):
    """Run the kernel on 8 cores; returns (output (32,4) f32, BassKernelResults)."""
    from concourse.bass_utils import run_bass_kernel_spmd

    nc = _get_nc()
    in_maps = _make_in_maps(Q_hid, D_hid, W, d_mask)
    res = run_bass_kernel_spmd(
        nc, in_maps, core_ids=list(range(N_CORES)), trace=trace, tmpdir=tmpdir
    )
    out = np.concatenate(
        [res.results[c]["out"].reshape(B_PER, N_P) for c in range(N_CORES)], axis=0
    ).astype(np.float32)
    return out, res


def kernel(Q_hid, D_hid, W, d_mask):
    out, _ = run_spmd(Q_hid, D_hid, W, d_mask, trace=False)
    return out

